# revision 23
# baseline (speedup 1.0000x reference)
"""AdaptiveFusionDecoder Trainium2 kernel (8 NeuronCores, SPMD, no collectives).

v4 strategy (vs the 897us fp8-DoubleRow baseline):
  - HW probe (probe_mm.py): at N=32 a plain bf16/fp16 [128,128]x[128,32]
    matmul streams at ~34ns, while fp8 DoubleRow [128,2,128]x[128,2,32]
    costs ~127ns (DR disables fast-weight-load; its LDWEIGHTS dominates).
    At N=500 DR gives NO streaming win (211ns either way).
  - So the recurrence gh = W_hh @ h_t runs as 192 fp16 matmuls/step
    (24 m-tiles x 8 k-chunks, N=32) ~ 6.5us/step, and the vocab-sharded
    logits run as fp16 1-pass units (8 x 211ns per 500-col block) instead
    of the fp8 3-pass scheme (12 matmuls). No fp8 anywhere -> no per-step
    h-split ops, shorter critical path, and rel err ~1.4e-3 predicted by
    simq.py (baseline measured 1.39e-2, budget 2e-2).
  - gh m-order r,z (PSUM bank A: m 0..15) then n (bank B: m 16..23): bank
    A closes at matmul 128 so the r/z sigmoids overlap the n matmuls.
  - X = emb @ W_ihx.T per 4-step block (fp16, N=128 stream-bound),
    staged through DRAM, emitted into step-chain idle windows.
  - All step intermediates fp16; h kept in one 16-slot fp16 ring used by
    alpha, gh, the gate math and the logits units.
"""

import os

os.environ.setdefault("MYCRO_LOCAL_CACHE", "1")

import numpy as np
import ml_dtypes
from contextlib import ExitStack

import concourse.bass as bass
import concourse.bacc as bacc
import concourse.tile as tile
from concourse import mybir
from concourse.bass_utils import run_bass_kernel_spmd
from concourse.tile import add_dep_helper


DEP_CLASSES = set(os.environ.get("KERNEL_DEPS", "none").split(","))


def _dep(from_i, to_i, reason, cls="misc"):
    """Force an explicit sync dependency between two emitted instructions."""
    if cls in DEP_CLASSES:
        add_dep_helper(from_i.ins, to_i.ins, sync=True, reason=reason)

V, E, H = 32000, 512, 1024
B, S = 32, 64
SB = S * B            # 2048 rows in step-major order: j = s*B + b
NCORES = 8
VS = V // NCORES      # 4000 vocab columns per core
H3 = 3 * H            # 3072
KH = H // 128         # 8 contraction chunks over H
KE = E // 128         # 4 contraction chunks over E
MH3 = H3 // 128       # 24 output tiles over 3H
NB_E = 8              # vocab n-blocks in the logits phase
NE = VS // NB_E       # 500 columns per logits matmul
MT_E = SB // 128      # 16 m-tiles (of 4 steps x 32 batch) in logits phase

F16 = mybir.dt.float16
F32 = mybir.dt.float32
npf16 = np.float16
AF = mybir.ActivationFunctionType

_CACHE = {}
LAST_RESULT = None
MARKERS = []


def _mark(nc, label):
    MARKERS.append((label, int(nc.get_next_instruction_name().split("-")[1])))


def _cm(a):
    """[K, M] -> chunk-major [128, (K//128)*M]; slice [:, k*M+m0 : k*M+m1]
    is rows k*128..(k+1)*128 of `a`, cols m0:m1 (a TensorE lhsT tile)."""
    a = np.asarray(a)
    K, M = a.shape
    kc = K // 128
    assert kc * 128 == K
    return np.ascontiguousarray(
        a.reshape(kc, 128, M).transpose(1, 0, 2).reshape(128, kc * M)
    )


def _gm(v):
    """[K] -> gate-major [128, K//128]: out[p, c] = v[c*128+p]"""
    v = np.asarray(v)
    K = v.shape[0]
    return np.ascontiguousarray(v.reshape(K // 128, 128).T)


def _bc(ap_, pos, count):
    """Insert a stride-0 (broadcast) free dim at free-position `pos`."""
    l = [list(x) for x in ap_.ap]
    l.insert(pos + 1, [0, count])
    return bass.AP(tensor=ap_.tensor, offset=ap_.offset, ap=l)


def build():
    nc = bacc.Bacc()

    # ---- parameters (per-core) ----
    embT_h = nc.declare_dram_parameter("embT", [128, KE * SB], F16, isOutput=False)
    imgT_h = nc.declare_dram_parameter("imgT", [128, KE * B], F32, isOutput=False)
    retT_h = nc.declare_dram_parameter("retT", [128, KE * B], F32, isOutput=False)
    wihx_h = nc.declare_dram_parameter("w_ihxT", [128, KE * H3], F16, isOutput=False)
    wihf_h = nc.declare_dram_parameter("w_ihfT", [128, KE * H3], F16, isOutput=False)
    whh_h = nc.declare_dram_parameter("whhT", [128, KH * H3], F16, isOutput=False)
    ihw_h = nc.declare_dram_parameter("init_hWT", [128, KE * H], F16, isOutput=False)
    ihb_h = nc.declare_dram_parameter("init_hbT", [128, KH], F32, isOutput=False)
    wgh_h = nc.declare_dram_parameter("w_ghT", [128, KH], F16, isOutput=False)
    wgi_h = nc.declare_dram_parameter("w_giT", [128, KE], F16, isOutput=False)
    wgr_h = nc.declare_dram_parameter("w_grT", [128, KE], F16, isOutput=False)
    gateb_h = nc.declare_dram_parameter("gate_b", [1, 1], F32, isOutput=False)
    bih_h = nc.declare_dram_parameter("b_ihT", [128, MH3], F32, isOutput=False)
    bhh_h = nc.declare_dram_parameter("b_hhT", [128, MH3], F32, isOutput=False)
    outw_h = nc.declare_dram_parameter("outWT", [128, KH * VS], F16, isOutput=False)
    out_h = nc.declare_dram_parameter("out", [SB, VS], F16, isOutput=True)

    with tile.TileContext(nc) as tc, ExitStack() as ctx:
        singles = ctx.enter_context(tc.tile_pool(name="singles", bufs=1))

        # ---- persistent SBUF tensors ----
        whh_sb = singles.tile([128, KH * H3], F16)           # 48KB/partition
        h16 = singles.tile([128, KH, 16, B], F16)            # h ring, slot t%16
        D3T = singles.tile([128, MH3, B], F16)
        R3b_f16 = singles.tile([128, MH3, B], F16)
        c_aT = singles.tile([1, B], F32)

        imgT_sb = singles.tile([128, KE, B], F32)
        retT_sb = singles.tile([128, KE, B], F32)
        imgT_f = singles.tile([128, KE, B], F16)
        retT_f = singles.tile([128, KE, B], F16)
        dT_f = singles.tile([128, KE, B], F16)
        ihb_sb = singles.tile([128, KH], F32)
        bih_sb = singles.tile([128, MH3], F32)
        bhh_sb = singles.tile([128, MH3], F32)
        bhhn_f = singles.tile([128, KH], F16)
        wgh_sb = singles.tile([128, KH], F16)
        wgi_sb = singles.tile([128, KE], F16)
        wgr_sb = singles.tile([128, KE], F16)
        gateb_sb = singles.tile([1, 1], F32)
        ones_f = singles.tile([1, 128], F16)
        one1_f = singles.tile([1, 1], F16)
        c_a_f = singles.tile([1, B], F16)

        nc.sync.dma_start(out=imgT_sb[:, :, :], in_=imgT_h[:, :].rearrange("p (c b) -> p c b", c=KE))
        nc.sync.dma_start(out=retT_sb[:, :, :], in_=retT_h[:, :].rearrange("p (c b) -> p c b", c=KE))
        nc.sync.dma_start(out=ihb_sb[:, :], in_=ihb_h[:, :])
        nc.sync.dma_start(out=bih_sb[:, :], in_=bih_h[:, :])
        nc.sync.dma_start(out=bhh_sb[:, :], in_=bhh_h[:, :])
        nc.sync.dma_start(out=wgh_sb[:, :], in_=wgh_h[:, :])
        nc.sync.dma_start(out=wgi_sb[:, :], in_=wgi_h[:, :])
        nc.sync.dma_start(out=wgr_sb[:, :], in_=wgr_h[:, :])
        nc.sync.dma_start(out=gateb_sb[:, :], in_=gateb_h[:, :])
        nc.vector.memset(ones_f[:, :], 1.0)
        nc.vector.memset(one1_f[:, :], 1.0)
        nc.vector.tensor_copy(imgT_f[:, :, :], imgT_sb[:, :, :])
        nc.vector.tensor_copy(retT_f[:, :, :], retT_sb[:, :, :])
        nc.vector.tensor_sub(dT_f[:, :, :], imgT_sb[:, :, :], retT_sb[:, :, :])
        nc.vector.tensor_copy(bhhn_f[:, :], bhh_sb[:, 16:24])

        # Cross-engine settling chain: serial Vector->Scalar->Tensor->...
        # ping-pong at kernel start. First-execution-in-a-process runs have
        # been observed to corrupt early cross-engine handoffs; this chain
        # absorbs any startup semaphore glitches before real work depends
        # on cross-engine ordering. (~3us once.)
        warm_a = singles.tile([128, 32], F16)
        warm_b = singles.tile([128, 32], F16)
        with tc.tile_pool(name="psW", bufs=1, space="PSUM") as psW:
            nc.vector.memset(warm_a[:, :], 0.25)
            for _ in range(3):
                nc.scalar.activation(out=warm_b[:, :], in_=warm_a[:, :], func=AF.Copy)
                pw = psW.tile([32, 32], F32, tag="pw")
                nc.tensor.matmul(out=pw[:, :], lhsT=warm_b[:, :], rhs=warm_b[:, :], start=True, stop=True)
                nc.scalar.activation(out=warm_b[0:32, :], in_=pw[:, :], func=AF.Copy, scale=1.0 / 64.0)
                nc.vector.tensor_add(warm_a[0:32, :], warm_a[0:32, :], warm_b[0:32, :])

        # W_hh DMA first: it gates step 0's gh and streams during phase A/C.
        _mark(nc, "wdma")
        nc.sync.dma_start(out=whh_sb[:, :], in_=whh_h[:, :])
        outw_sb = ctx.enter_context(tc.tile_pool(name="outw", bufs=1)).tile([128, KH, VS], F16)

        # ======== Phase A, replicated: every core computes ALL 16 X-blocks.
        # Blocks live in a 3-slot SBUF ring (produced 2 blocks ahead of
        # consumption); Tile tracks the slot reuse, so production/consumption
        # stay ordered without a DRAM round trip.
        embp = ctx.enter_context(tc.tile_pool(name="embp", bufs=3))
        wihx_sb = ctx.enter_context(tc.tile_pool(name="wihx", bufs=1)).tile([128, KE * H3], F16)
        nc.sync.dma_start(out=wihx_sb[:, :], in_=wihx_h[:, :])
        stg_pool = ctx.enter_context(tc.tile_pool(name="stg", bufs=3))
        psB = ctx.enter_context(tc.tile_pool(name="psB", bufs=2, space="PSUM"))
        xblocks = {}

        def emit_xblock(bl, fold=True):
            _mark(nc, f"phaseA_bl{bl}")
            embT = embp.tile([128, KE, 128], F16, tag="embT")
            nc.sync.dma_start(
                out=embT[:, :, :],
                in_=embT_h[:, :].rearrange("p (c j) -> p c j", c=KE)[:, :, bl * 128 : (bl + 1) * 128],
            )
            # X block for steps 4bl..4bl+3: [128][m:24][s4:4][b:32]
            stg = stg_pool.tile([128, MH3, 4, B], F16, tag="stg")
            for mp in range(MH3 // 2):
                px = psB.tile([128, 2, 128], F32, tag="psb")
                for half in range(2):
                    m = 2 * mp + half
                    for k in range(KE):
                        nc.tensor.matmul(
                            out=px[:, half, :],
                            lhsT=wihx_sb[:, k * H3 + m * 128 : k * H3 + (m + 1) * 128],
                            rhs=embT[:, k, :],
                            start=(k == 0),
                            stop=(k == KE - 1),
                        )
                nc.scalar.activation(
                    out=stg[:, 2 * mp : 2 * mp + 2, :, :],
                    in_=px[:, :, :].rearrange("p h (s b) -> p h s b", s=4),
                    func=AF.Copy,
                )
            # fold the step-invariant R3/bias terms in place, once per block
            # (skipped for the sacrificial warmup block: R3b isn't written yet
            # and the read would WAR-serialize phase C behind the warmup)
            if fold:
                nc.vector.tensor_add(stg[:, :, :, :], stg[:, :, :, :], _bc(R3b_f16[:, :, :], 1, 4))
            xblocks[bl] = stg

        # Warmup production of block 0, re-emitted for real after phase C.
        # The first block produced at kernel start (cold icache, every weight
        # DMA in flight) has been observed to corrupt on first execution in
        # a process; the re-emission overwrites it in a calm window. (The
        # original baseline's duplicate block-0 emission served the same
        # purpose.)
        emit_xblock(0, fold=False)

        _mark(nc, "phaseC")

        # ======== Phase C: h0, c_a, R3/D3 ========
        # Emitted twice: once early (results possibly corrupted by the
        # first-execution startup window), once again in a calm window
        # before the steps consume them; the second write wins.
        def phase_c(tag):
          with ExitStack() as cctx:
            R3b = cctx.enter_context(tc.tile_pool(name="r3bp" + tag, bufs=1)).tile([128, MH3, B], F32)
            wihf_sb = cctx.enter_context(tc.tile_pool(name="wihf" + tag, bufs=1)).tile([128, KE * H3], F16)
            nc.sync.dma_start(out=wihf_sb[:, :], in_=wihf_h[:, :])
            ihw_sb = cctx.enter_context(tc.tile_pool(name="ihw" + tag, bufs=1)).tile([128, KE * H], F16)
            nc.sync.dma_start(out=ihw_sb[:, :], in_=ihw_h[:, :])
            psC1 = cctx.enter_context(tc.tile_pool(name="psC1" + tag, bufs=1, space="PSUM"))
            psC2 = cctx.enter_context(tc.tile_pool(name="psC2" + tag, bufs=1, space="PSUM"))
            psC3 = cctx.enter_context(tc.tile_pool(name="psC3" + tag, bufs=2, space="PSUM"))

            # h0 = tanh(init_h_W @ img^T + b)  -> h16 slot 15 (t=-1)
            ph0 = psC1.tile([128, KH, B], F32)
            for m in range(KH):
                for k in range(KE):
                    nc.tensor.matmul(
                        out=ph0[:, m, :],
                        lhsT=ihw_sb[:, k * H + m * 128 : k * H + (m + 1) * 128],
                        rhs=imgT_f[:, k, :],
                        start=(k == 0),
                        stop=(k == KE - 1),
                    )
            i_h0 = None
            for m in range(KH):
                i_h0 = nc.scalar.activation(out=h16[:, m, 15, :], in_=ph0[:, m, :], func=AF.Tanh, bias=ihb_sb[:, m : m + 1])

            # c_a = img @ w_gi + ret @ w_gr + gate_b
            pca = psC2.tile([1, B], F32)
            for k in range(KE):
                nc.tensor.matmul(out=pca[:, :], lhsT=wgi_sb[:, k : k + 1], rhs=imgT_f[:, k, :], start=(k == 0), stop=False)
            for k in range(KE):
                nc.tensor.matmul(out=pca[:, :], lhsT=wgr_sb[:, k : k + 1], rhs=retT_f[:, k, :], start=False, stop=(k == KE - 1))
            gb = gateb_sb[:, :]
            nc.vector.tensor_add(c_aT[:, :], pca[:, :], bass.AP(tensor=gb.tensor, offset=gb.offset, ap=[list(gb.ap[0]), [0, B]]))
            i_caf = nc.vector.tensor_copy(c_a_f[:, :], c_aT[:, :])

            # R3^T and D3^T
            for m in range(MH3):
                pr = psC3.tile([128, B], F32, tag="psc3")
                for k in range(KE):
                    nc.tensor.matmul(
                        out=pr[:, :],
                        lhsT=wihf_sb[:, k * H3 + m * 128 : k * H3 + (m + 1) * 128],
                        rhs=retT_f[:, k, :],
                        start=(k == 0),
                        stop=(k == KE - 1),
                    )
                nc.scalar.activation(out=R3b[:, m, :], in_=pr[:, :], func=AF.Copy)
                pd = psC3.tile([128, B], F32, tag="psc3")
                for k in range(KE):
                    nc.tensor.matmul(
                        out=pd[:, :],
                        lhsT=wihf_sb[:, k * H3 + m * 128 : k * H3 + (m + 1) * 128],
                        rhs=dT_f[:, k, :],
                        start=(k == 0),
                        stop=(k == KE - 1),
                    )
                i_d3 = nc.scalar.activation(out=D3T[:, m, :], in_=pd[:, :], func=AF.Copy)
            # fold biases into R3b:  + b_ih everywhere, + b_hh on the r,z chunks
            nc.vector.tensor_add(R3b[:, :, :], R3b[:, :, :], _bc(bih_sb[:, :], 1, B))
            nc.vector.tensor_add(R3b[:, 0:16, :], R3b[:, 0:16, :], _bc(bhh_sb[:, 0:16], 1, B))
            nc.vector.tensor_copy(R3b_f16[:, :, :], R3b[:, :, :])
            return i_h0, i_caf, i_d3

        phase_c("a")
        i_h0, i_caf, i_d3 = phase_c("b")

        # outW DMA issued after the phase A/C weight loads it would otherwise
        # compete with; first consumer is the first logits unit (t=4).
        nc.sync.dma_start(out=outw_sb[:, :, :], in_=outw_h[:, :].rearrange("p (k v) -> p k v", k=KH))

        emit_xblock(1)
        emit_xblock(0)

        # ======== Phase D: recurrence with interleaved logits units ========
        gp = ctx.enter_context(tc.tile_pool(name="gp", bufs=2))
        psD_gh = ctx.enter_context(tc.tile_pool(name="psDgh", bufs=1, space="PSUM"))
        psD_a = ctx.enter_context(tc.tile_pool(name="psDa", bufs=1, space="PSUM"))
        psD_ab = ctx.enter_context(tc.tile_pool(name="psDab", bufs=1, space="PSUM"))
        psE = ctx.enter_context(tc.tile_pool(name="psE", bufs=2, space="PSUM"))
        stE = ctx.enter_context(tc.tile_pool(name="stE", bufs=2))

        units = [(m, nb) for m in range(MT_E) for nb in range(NB_E)]
        ui = 0

        def emit_unit(m, nb):
            # logits block: PSUM = h-slots(4 steps) @ outW, fp16 1-pass
            pe = psE.tile([128, NE], F32, tag="pse")
            sl = (4 * m) % 16
            n0 = nb * NE
            for k in range(KH):
                nc.tensor.matmul(
                    out=pe[:, :],
                    lhsT=h16[:, k, sl : sl + 4, :],
                    rhs=outw_sb[:, k, n0 : n0 + NE],
                    start=(k == 0),
                    stop=(k == KH - 1),
                )
            st = stE.tile([128, NE], F16, tag="ste")
            nc.scalar.activation(out=st[:, :], in_=pe[:, :], func=AF.Copy)
            nc.sync.dma_start(out=out_h[m * 128 : (m + 1) * 128, nb * NE : (nb + 1) * NE], in_=st[:, :])

        xrt4 = None
        for t in range(S):
            _mark(nc, f"step{t}")
            sl_t = (t + 15) % 16   # h_t slot
            sl_n = (t + 16) % 16   # h_{t+1} slot
            if t % 4 == 0:
                xrt4 = xblocks.pop(t // 4)

            # alpha_t = sigmoid(h_t . w_gh + c_a)  (c_a folded into the PSUM acc)
            pa = psD_a.tile([1, B], F32, tag="pa")
            i_alpha0 = nc.tensor.matmul(out=pa[:, :], lhsT=one1_f[:, :], rhs=c_a_f[:, :], start=True, stop=False)
            if t == 0:
                _dep(i_h0, i_alpha0, "h0 tanh -> step0 tensor", cls="step0")
                _dep(i_caf, i_alpha0, "c_a_f -> step0 tensor", cls="step0")
            for k in range(KH):
                nc.tensor.matmul(out=pa[:, :], lhsT=wgh_sb[:, k : k + 1], rhs=h16[:, k, sl_t, :], start=False, stop=(k == KH - 1))
            al = gp.tile([1, B], F16, tag="al")
            nc.scalar.activation(out=al[:, :], in_=pa[:, :], func=AF.Sigmoid)

            # gh^T: PSUM = W_hh @ h_t, fp16 1-pass.
            # m-order r,z (PSUM bank A: m 0..15) then n (bank B: m 16..23):
            # bank A closes early so the r/z sigmoids overlap the n matmuls.
            pgh = psD_gh.tile([128, MH3, B], F32, tag="pgh")
            first_m = True
            for m in range(MH3):
                for k in range(KH):
                    nc.tensor.matmul(
                        out=pgh[:, m, :],
                        lhsT=whh_sb[:, k * H3 + m * 128 : k * H3 + (m + 1) * 128],
                        rhs=h16[:, k, sl_t, :],
                        start=(k == 0),
                        stop=(k == KH - 1),
                    )
                if first_m:
                    # alpha broadcast right after the first m-group: its
                    # sigmoid-wait hides behind the remaining gh groups
                    pab = psD_ab.tile([128, B], F32, tag="pab")
                    nc.tensor.matmul(out=pab[:, :], lhsT=ones_f[:, :], rhs=al[:, :], start=True, stop=True)
                    gi = gp.tile([128, MH3, B], F16, tag="u")
                    i_gimul = nc.vector.tensor_mul(gi[:, :, :], D3T[:, :, :], _bc(pab[:, :], 0, MH3))
                    if t == 0:
                        _dep(i_d3, i_gimul, "D3T copies -> step0 gi mul", cls="step0")
                        _dep(i_h0, i_gimul, "h0 tanh -> step0 vector", cls="step0")
                    nc.vector.tensor_add(gi[:, :, :], gi[:, :, :], xrt4[:, :, t % 4, :])
                    first_m = False

            # r = sigmoid(gi_r + gh_r)   (bank A complete after m=15)
            r_s = gp.tile([128, KH, B], F16, tag="rs")
            nc.vector.tensor_add(r_s[:, :, :], pgh[:, 0:8, :], gi[:, 0:8, :])
            nc.scalar.activation(out=r_s[:, :, :], in_=r_s[:, :, :], func=AF.Sigmoid)

            # z = sigmoid(gi_z + gh_z)  (off the n critical path, overlaps n MMs)
            z_s = gp.tile([128, KH, B], F16, tag="zs")
            nc.vector.tensor_add(z_s[:, :, :], pgh[:, 8:16, :], gi[:, 8:16, :])
            nc.scalar.activation(out=z_s[:, :, :], in_=z_s[:, :, :], func=AF.Sigmoid)

            # n = tanh(gi_n + r*(gh_n + b_hh_n))
            hn = gp.tile([128, KH, B], F16, tag="hn")
            nc.vector.tensor_add(hn[:, :, :], pgh[:, 16:24, :], _bc(bhhn_f[:, :], 1, B))
            t1 = gp.tile([128, KH, B], F16, tag="t1")
            nc.vector.tensor_mul(t1[:, :, :], r_s[:, :, :], hn[:, :, :])
            t2 = gp.tile([128, KH, B], F16, tag="t2")
            nc.vector.tensor_add(t2[:, :, :], t1[:, :, :], gi[:, 16:24, :])
            nn = gp.tile([128, KH, B], F16, tag="nn")
            nc.scalar.activation(out=nn[:, :, :], in_=t2[:, :, :], func=AF.Tanh)

            # h_new = n + z*(h - n)  -> h16 slot t%16
            hmn = gp.tile([128, KH, B], F16, tag="hmn")
            nc.vector.tensor_sub(hmn[:, :, :], h16[:, :, sl_t, :], nn[:, :, :])
            nc.vector.tensor_mul(hmn[:, :, :], z_s[:, :, :], hmn[:, :, :])
            nc.vector.tensor_add(h16[:, :, sl_n, :], nn[:, :, :], hmn[:, :, :])

            # produce X block t//4+2 during this block's chain-idle windows
            if t % 4 == 2 and t // 4 + 2 < 16:
                emit_xblock(t // 4 + 2)

            # interleave logits units whose h slots are final; keep 2/step in
            # the middle and 3/step late so the last steps' chain windows
            # (no X blocks left to produce) still have TensorE fill.
            budget = 0 if t < 4 else (3 if t >= 56 else 2)
            while budget > 0 and ui < len(units):
                m, nb = units[ui]
                if 4 * m + 4 > t:
                    break
                emit_unit(m, nb)
                ui += 1
                budget -= 1

        # tail: remaining logits units
        _mark(nc, "tail")
        while ui < len(units):
            m, nb = units[ui]
            emit_unit(m, nb)
            ui += 1

    nc.finalize()
    return nc


def _prep_inputs(inputs):
    inp = {k: np.asarray(v) for k, v in inputs.items()}
    tokens = inp["tokens_in"].astype(np.int64)                  # [B, S]
    tok_sm = np.ascontiguousarray(tokens.T).reshape(SB)         # j = s*B + b
    emb = np.asarray(inp["tok_embed"], np.float32)[tok_sm]      # [SB, E]
    embT = _cm(emb.T.astype(np.float32))                        # [128, KE*SB]

    W_ih = inp["gru_W_ih"].astype(np.float32)                   # [3H, 2E]
    gw = inp["gate_W"].astype(np.float32)[0]                    # [H + 2E]
    whhT = np.ascontiguousarray(inp["gru_W_hh"].astype(np.float32).T)  # [H, 3H]
    outW = inp["out_W"].astype(np.float32)

    common = {
        "embT": embT.astype(npf16),
        "imgT": _cm(inp["image_emb"].astype(np.float32).T),
        "retT": _cm(inp["retrieved_emb"].astype(np.float32).T),
        "w_ihxT": _cm(W_ih[:, :E].T).astype(npf16),
        "w_ihfT": _cm(W_ih[:, E:].T).astype(npf16),
        "whhT": _cm(whhT).astype(npf16),
        "init_hWT": _cm(inp["init_h_W"].astype(np.float32).T).astype(npf16),
        "init_hbT": _gm(inp["init_h_b"]).astype(np.float32),
        "w_ghT": _gm(gw[:H]).astype(npf16),
        "w_giT": _gm(gw[H : H + E]).astype(npf16),
        "w_grT": _gm(gw[H + E :]).astype(npf16),
        "gate_b": inp["gate_b"].astype(np.float32).reshape(1, 1),
        "b_ihT": _gm(inp["gru_b_ih"]).astype(np.float32),
        "b_hhT": _gm(inp["gru_b_hh"]).astype(np.float32),
    }

    in_maps = []
    for c in range(NCORES):
        m = dict(common)
        wt = np.ascontiguousarray(outW[c * VS : (c + 1) * VS].T)  # [H, VS]
        m["outWT"] = _cm(wt).astype(npf16)
        in_maps.append(m)
    return in_maps


def kernel(**inputs):
    global LAST_RESULT
    if "nc" not in _CACHE:
        _CACHE["nc"] = build()
    nc = _CACHE["nc"]
    in_maps = _prep_inputs(inputs)
    trace = bool(int(os.environ.get("KERNEL_TRACE", "0")))
    res = run_bass_kernel_spmd(nc, in_maps, core_ids=list(range(NCORES)), trace=trace)
    LAST_RESULT = res
    full = np.concatenate(
        [np.asarray(res.results[c]["out"]).astype(np.float32) for c in range(NCORES)], axis=1
    )
    full += np.asarray(inputs["out_b"]).astype(np.float32)[None, :]
    return np.ascontiguousarray(full.reshape(S, B, V).transpose(1, 0, 2))
